# revision 5
# baseline (speedup 1.0000x reference)
"""Trainium2 Bass kernel for nn_EuclideanEmbedding (fused cutoff-multiply +
segment_sum over 3.2M edges into 100k nodes, 16 features).

Strategy (v2 — node-per-partition layout, no PE scatter)
--------------------------------------------------------
Host: drop edges with r >= R_CUT (w == 0 exactly), sort nodes by degree,
assign each node to one (core, slot, partition) cell; pack each node's
edges contiguously along the SBUF free axis, padded to a per-tile uniform
capacity CB_t (degree sorting keeps padding small). Layout per tile:
x[p, slot, f, c] (f-outer, c-inner, bf16) and l[p, slot, c].

Device (per core): per tile — DMA x/l, scalar engine computes
w = 0.5*INV_AVG*(cos(pi*l/R_CUT)+1) (Sin + Copy activations), one DVE
tensor_tensor multiply x *= w (broadcast over f), one DVE
tensor_reduce(axis=X) over c producing [128, m*16] f32 directly into the
output accumulator. The segment-sum is a contiguous-axis reduction; the
tensor engine is not used at all.

Output rows >= 100000 of the full [3.2M, 16] result are identically zero
(receivers < 100000), assembled host-side with the inverse node permute.
"""
import math

import numpy as np
import ml_dtypes

E = 3_200_000
F = 16
N_NODES = 100_000
R_CUT = 5.0
INV_AVG = 1.0 / 32.0
K_W = 0.5 * INV_AVG

N_CORES = 8
P = 128                     # nodes per bucket (one SBUF partition each)
SPT = 8                     # slots (buckets) per tile — cap-quantization unit

_NBUCK = (N_NODES + P - 1) // P                   # 782 real buckets
BUCKETS = (_NBUCK + N_CORES - 1) // N_CORES * N_CORES   # 784 global buckets
SLOTS = BUCKETS // N_CORES                        # 98 per core
NODES_PAD = BUCKETS * P                           # 100352
TILES = -(-SLOTS // SPT)                          # 13 (last tile has 2 slots)

_CACHE = {}


def _build_program(caps, reps: int = 1):
    """caps: per-tile uniform edge capacity (tuple of TILES ints)."""
    from contextlib import ExitStack

    import concourse.bacc as bacc
    import concourse.tile as tile
    from concourse import mybir

    tile_slots = [min(SPT, SLOTS - t * SPT) for t in range(TILES)]
    l_cols = [m * cb for m, cb in zip(tile_slots, caps)]
    l_off = np.concatenate([[0], np.cumsum(l_cols)]).astype(int)
    totc = int(l_off[-1])

    nc = bacc.Bacc("TRN2", target_bir_lowering=False, debug=False,
                   enable_asserts=False, num_devices=N_CORES)
    x_dram = nc.dram_tensor("x_t", [P, totc * F], mybir.dt.bfloat16,
                            kind="ExternalInput").ap()
    l_dram = nc.dram_tensor("l_t", [P, totc], mybir.dt.bfloat16,
                            kind="ExternalInput").ap()
    out_dram = nc.dram_tensor("out", [P, SLOTS * F], mybir.dt.float32,
                              kind="ExternalOutput").ap()

    with tile.TileContext(nc) as tc, ExitStack() as ctx:
        small = ctx.enter_context(tc.tile_pool(name="small", bufs=1))
        xin = ctx.enter_context(tc.tile_pool(name="xin", bufs=3))
        lin = ctx.enter_context(tc.tile_pool(name="lin", bufs=3))
        wrk = ctx.enter_context(tc.tile_pool(name="wrk", bufs=3))

        out_sbuf = small.tile([P, SLOTS * F], mybir.dt.float32)
        halfpi = small.tile([P, 1], mybir.dt.float32)
        nc.gpsimd.memset(halfpi[:], math.pi / 2)

        for _rep in range(reps):
            for t in range(TILES):
                m, cb = tile_slots[t], caps[t]
                nl = m * cb
                lt = lin.tile([P, nl], mybir.dt.bfloat16, tag="lt")
                nc.sync.dma_start(lt[:], l_dram[:, l_off[t]:l_off[t] + nl])
                xt = xin.tile([P, nl * F], mybir.dt.bfloat16, tag="xt")
                nc.sync.dma_start(xt[:], x_dram[:, l_off[t] * F:
                                                (l_off[t] + nl) * F])

                # w = K_W * (cos(pi*l/R_CUT) + 1); kept edges have l < R_CUT,
                # pad slots have l = R_CUT exactly -> w = 0.
                u = wrk.tile([P, nl], mybir.dt.float32, tag="u")
                nc.scalar.activation(u[:], lt[:],
                                     mybir.ActivationFunctionType.Sin,
                                     bias=halfpi[:, 0:1],
                                     scale=-math.pi / R_CUT)
                w = wrk.tile([P, nl], mybir.dt.bfloat16, tag="w")
                nc.scalar.activation(w[:], u[:],
                                     mybir.ActivationFunctionType.Copy,
                                     bias=K_W, scale=K_W)

                xv = xt[:].rearrange("p (s f c) -> p s f c", f=F, c=cb)
                wv = w[:].rearrange("p (s c) -> p s c", c=cb) \
                    .unsqueeze(2).broadcast_to([P, m, F, cb])
                nc.vector.tensor_tensor(xv, xv, wv, mybir.AluOpType.mult)
                nc.vector.tensor_reduce(
                    out_sbuf[:, t * SPT * F:(t * SPT + m) * F], xv,
                    mybir.AxisListType.X, mybir.AluOpType.add)

        nc.sync.dma_start(out_dram[:], out_sbuf[:])

    nc.compile()
    return nc


def _get_program(caps, reps: int = 1):
    key = (tuple(caps), reps)
    if key not in _CACHE:
        _CACHE[key] = _build_program(tuple(caps), reps)
    return _CACHE[key]


def _prepare(senders, lengths, receivers):
    """Filter + degree-sort + pack. Returns (in_maps, caps, order_pad)."""
    lengths = np.asarray(lengths, dtype=np.float32).reshape(-1)
    keep = lengths < R_CUT
    s_k = np.asarray(senders, dtype=np.float32)[keep]
    l_k = lengths[keep]
    r_k = np.asarray(receivers).astype(np.int64)[keep]

    deg = np.bincount(r_k, minlength=N_NODES)
    n_phantom = NODES_PAD - N_NODES
    # order_pad[rank] = node id (phantoms first, then nodes by ascending deg)
    order_pad = np.concatenate([
        np.arange(N_NODES, NODES_PAD, dtype=np.int64),
        np.argsort(deg, kind="stable").astype(np.int64)])
    rank_of = np.empty(NODES_PAD, np.int64)
    rank_of[order_pad] = np.arange(NODES_PAD)

    # per-tile capacity: buckets are ascending-degree, so the max degree in
    # tile t is the degree of the last node of its last bucket
    deg_pad = np.concatenate([deg, np.zeros(n_phantom, np.int64)])
    deg_sorted = deg_pad[order_pad]
    tile_slots = [min(SPT, SLOTS - t * SPT) for t in range(TILES)]
    caps, hi = [], 0
    for t in range(TILES):
        hi += tile_slots[t] * N_CORES * P
        caps.append(max(1, int(deg_sorted[:hi].max() if t == 0
                               else deg_sorted[hi - 1])))
    caps = [max(1, int(c)) for c in caps]
    l_cols = [m * cb for m, cb in zip(tile_slots, caps)]
    l_off = np.concatenate([[0], np.cumsum(l_cols)]).astype(np.int64)
    totc = int(l_off[-1])

    # per-edge placement
    rank = rank_of[r_k]                       # rank in sorted node list
    j = rank // P                             # global bucket
    p_idx = rank % P                          # partition row
    core = j % N_CORES
    slot = j // N_CORES
    t_id = slot // SPT
    cb_e = np.asarray(caps, np.int64)[t_id]
    col0 = l_off[t_id] + (slot - t_id * SPT) * cb_e   # node's l-col base

    # within-node edge index c: order edges by rank, then running index
    eorder = np.argsort(rank, kind="stable")
    cnt = np.bincount(rank, minlength=NODES_PAD)
    starts = np.concatenate([[0], np.cumsum(cnt)[:-1]])
    c_sorted = np.arange(len(r_k), dtype=np.int64) - starts[rank[eorder]]
    c = np.empty(len(r_k), np.int64)
    c[eorder] = c_sorted

    x_all = np.zeros((N_CORES, P, totc * F), ml_dtypes.bfloat16)
    l_all = np.full((N_CORES, P, totc), R_CUT, ml_dtypes.bfloat16)
    l_all[core, p_idx, col0 + c] = l_k.astype(ml_dtypes.bfloat16)
    s_bf = s_k.astype(ml_dtypes.bfloat16)
    xbase = col0 * F + c
    for f in range(F):
        x_all[core, p_idx, xbase + f * cb_e] = s_bf[:, f]

    in_maps = [{"x_t": x_all[k], "l_t": l_all[k]} for k in range(N_CORES)]
    return in_maps, caps, order_pad


def _run(inputs, trace=False, **run_kwargs):
    from concourse.bass_utils import run_bass_kernel_spmd

    in_maps, caps, order_pad = _prepare(
        inputs["senders"], inputs["lengths"], inputs["receivers"])
    nc = _get_program(caps)
    try:
        res = run_bass_kernel_spmd(nc, in_maps, core_ids=list(range(N_CORES)),
                                   trace=trace, **run_kwargs)
    except Exception:
        # transient NRT device wedges have been observed; one retry
        res = run_bass_kernel_spmd(nc, in_maps, core_ids=list(range(N_CORES)),
                                   trace=trace, **run_kwargs)

    # by_rank[j, p] = output row of node order_pad[j*P + p]
    by_rank = np.empty((BUCKETS, P, F), np.float32)
    for k in range(N_CORES):
        o = np.asarray(res.results[k]["out"], np.float32)  # [P, SLOTS*F]
        by_rank[k::N_CORES] = o.reshape(P, SLOTS, F).transpose(1, 0, 2)
    out_full = np.zeros((E, F), np.float32)
    flat = by_rank.reshape(NODES_PAD, F)
    real = order_pad < N_NODES
    out_full[order_pad[real]] = flat[real]
    return out_full, res, caps


def kernel(senders, lengths, vectors, receivers):
    out, _, _ = _run({"senders": senders, "lengths": lengths,
                      "receivers": receivers})
    return out
